# revision 1
# baseline (speedup 1.0000x reference)
"""Trainium2 Bass kernel for nn_MetaMultiLinear.

Math (per head h, sample b):
    w[b, k]   = sum_c cond[b, c] * CW[k, c] + cb[k]        k = o*17 + i  (544)
    out[b, o] = sum_i x1[b, i] * w[b, o*17+i]              x1 = [input, 1] (17)

Sharding: head h -> NeuronCore h (8 heads, 8 cores), full B=32768 per core.

Split i = 0..15 (needs the per-sample multiply) from i = 16 (x1 = 1, so its
contribution cond1 @ CWones^T + bias goes straight into the output
accumulator).

Per-core device algorithm (tiles of 128 samples, processed in pairs; the
group loop is a For_i hardware loop so the static program stays small —
this runtime's per-execution cost scales with static NEFF size):
  1. One DMA per group loads [cond|1|x|pad] for 2*GRP tiles.
  2. Per pair: one PE transpose (plus a 1x1 fence matmul that carries the
     semaphore waits — transpose-mode matmuls only take one sync wait)
     gives cond1^T at partitions 0-32 / 64-96; ScalarE copies PSUM->SBUF.
  3. Per tile (PE, float32r): W-MM  w1[b, o*16+i] = cond1 @ CWk^T  (K=33,
     N=512, one PSUM bank); po-MM  po[b, o] = cond1 @ CWones^T (start=True,
     opens the tile's accumulation group, carries all bias terms).
  4. Per tile (DVE, the floor: one 1x pass, 512 elem/partition): tmp =
     w1 (*) broadcast(x), reading w1 straight from PSUM.
  5. Per tile (PE, float32r): one reduce matmul with identity stationary
     streams tmp i-outer/o-inner; the PSUM out AP is a broadcast view so
     16 passes accumulate onto po[b, o] via has_written.
  6. ScalarE copies po -> SBUF (DMA cannot read PSUM); one output DMA per
     group. Reduce phases run one pair behind produce phases so the PE
     FIFO has W work while the DVE multiply runs.
"""

import sys

import numpy as np

if "/opt/trn_rl_repo" not in sys.path:
    sys.path.insert(0, "/opt/trn_rl_repo")

N_HEADS, IN_F, COND_IN, OUT_F = 8, 16, 32, 32
B = 32768
INP1 = IN_F + 1  # 17
KW = OUT_F * IN_F  # 512 (i<16 part)
C1 = COND_IN + 1  # 33
P = 128
GRP = 16  # pairs per group

_cached_nc = None

# "overlap": PE grouped reduce via overlapping PSUM out-AP (1 matmul/tile)
# "mm16":    PE grouped reduce via 16 accumulated strided matmuls (sim-safe)
REDUCE_MODE = "overlap"
# float32r: single-pass fast fp32 on PE (1 cycle/row at N>=256; exact fp32
# costs 4 cycles/row). Operands must be typed f32r at their producers.
USE_F32R = True
# use a For_i hardware loop over groups (small static program)
USE_LOOP = True


def _build_nc(b_total=B, grp=None, reps=1, loop=None):
    import concourse.bass as bass
    import concourse.mybir as mybir
    import concourse.tile as tile
    from concourse import bacc
    from contextlib import ExitStack

    f32 = mybir.dt.float32
    fr = mybir.dt.float32r if USE_F32R else f32
    if loop is None:
        loop = USE_LOOP
    nc = bacc.Bacc()
    pairs = b_total // (2 * P)
    if grp is None:
        grp = GRP
    while pairs % grp:
        grp //= 2
    groups = pairs // grp
    gsz = 2 * grp * P  # samples per group

    # cx: per sample [cond (32) | 1.0 | input (16) | zeros (15)]
    cx_t = nc.dram_tensor("cx", [b_total, 64], f32, kind="ExternalInput")
    # cwk[c, o*16+i] = CW[o*17+i, c] (i<16); row 32 = cond_bias slice
    cwk_t = nc.dram_tensor("cwk", [P, KW], fr, kind="ExternalInput")
    # cwo[c, o] = CW[o*17+16, c]; row 32 = cond_bias[o*17+16]
    cwo_t = nc.dram_tensor("cwo", [P, OUT_F], fr, kind="ExternalInput")
    ident_t = nc.dram_tensor("ident", [P, P], fr, kind="ExternalInput")
    out_t = nc.dram_tensor("out", [b_total, OUT_F], f32, kind="ExternalOutput")

    with tile.TileContext(nc) as tc, ExitStack() as ctx:
        consts = ctx.enter_context(tc.tile_pool(name="consts", bufs=1))
        ptrin = ctx.enter_context(tc.tile_pool(name="ptrin", bufs=2))
        ptrs = ctx.enter_context(tc.tile_pool(name="ptrs", bufs=4))
        ptmp = ctx.enter_context(tc.tile_pool(name="ptmp", bufs=4))
        pouts = ctx.enter_context(tc.tile_pool(name="pouts", bufs=2))
        pps_tr = ctx.enter_context(tc.tile_pool(name="pps_tr", bufs=1, space="PSUM"))
        pps_w = ctx.enter_context(tc.tile_pool(name="pps_w", bufs=3, space="PSUM"))
        pps_o = ctx.enter_context(tc.tile_pool(name="pps_o", bufs=2, space="PSUM"))

        cwk = consts.tile([P, KW], fr)
        nc.sync.dma_start(out=cwk, in_=cwk_t[:])
        cwo = consts.tile([P, OUT_F], fr)
        nc.sync.dma_start(out=cwo, in_=cwo_t[:])
        idn = consts.tile([P, P], fr)
        nc.sync.dma_start(out=idn, in_=ident_t[:])
        idn32 = idn.bitcast(f32)

        def emit_group(gb0):
            """Emit one group's program. gb0: starting sample (int or reg)."""
            trin_g = ptrin.tile([P, 2 * grp, 64], f32)
            nc.sync.dma_start(
                out=trin_g[:],
                in_=cx_t[bass.ds(gb0, gsz), :].rearrange(
                    "(t p) c -> p t c", t=2 * grp
                ),
            )
            outs_g = pouts.tile([P, 2 * grp, OUT_F], f32)

            pending = []  # (po, tmps, col)

            def emit_reduce(item):
                po, tmps, col = item
                for t in (0, 1):
                    tmp = tmps[t]
                    if REDUCE_MODE == "overlap":
                        # 16 streamed passes of 32 o-columns accumulate onto
                        # the same PSUM addresses via has_written. i-outer/
                        # o-inner keeps the dst innermost step-1/even/8B-
                        # aligned (fp32r paired PSUM write requirement).
                        rhs = tmp[:].rearrange("p o i -> p i o")
                        ov = (
                            po[:, t, 0:OUT_F]
                            .unsqueeze(1)
                            .broadcast_to([P, IN_F, OUT_F])
                        )
                        nc.tensor.matmul(
                            ov,
                            idn[:],
                            rhs,
                            start=False,
                            stop=True,
                            skip_group_check=True,
                        )
                    else:
                        tv = tmp[:].rearrange("p o i -> p i o")
                        for i in range(IN_F):
                            nc.tensor.matmul(
                                po[:, t, 0:OUT_F],
                                idn[:],
                                tv[:, i, :],
                                start=False,
                                stop=(i == IN_F - 1),
                                skip_group_check=True,
                            )
                # PSUM -> SBUF (DMA cannot read PSUM)
                nc.scalar.copy(out=outs_g[:, col : col + 2, :], in_=po[:, :, 0:OUT_F])

            for pr in range(grp):
                trin = trin_g[:, 2 * pr : 2 * pr + 2, :].rearrange("p t c -> p (t c)")
                trps = pps_tr.tile([P, P], f32)
                # Fence: carries the semaphore waits (trin DMA, idn DMA,
                # trps slot release); transpose-mode matmuls only support a
                # single sync-wait in codegen.
                nc.tensor.matmul(
                    trps[0:1, 0:1],
                    trin[:, 0:1],
                    idn32[:, 0:1],
                    start=True,
                    stop=True,
                    skip_group_check=True,
                )
                nc.tensor.transpose(trps[:], trin[:], idn32[:])
                trs = ptrs.tile([P, P], fr)
                nc.scalar.copy(out=trs[:], in_=trps[:])

                po = pps_o.tile([P, 2, 512], f32)
                tmps = []
                for t in (0, 1):
                    g = t * 64
                    cts = trs[g : g + C1, :]
                    w1 = pps_w.tile([P, KW], f32)
                    nc.tensor.matmul(
                        w1[:],
                        cts,
                        cwk[g : g + C1, :],
                        start=True,
                        stop=True,
                        tile_position=(g, 0),
                    )
                    # opens tile t's accumulation group (own PSUM bank)
                    nc.tensor.matmul(
                        po[:, t, 0:OUT_F],
                        cts,
                        cwo[g : g + C1, :],
                        start=True,
                        stop=False,
                        skip_group_check=True,
                        tile_position=(g, 0),
                    )
                    tmp = ptmp.tile([P, OUT_F, IN_F], fr)
                    w1v = w1[:].rearrange("p (o i) -> p o i", i=IN_F)
                    xv = (
                        trin[:, g + C1 : g + C1 + IN_F]
                        .unsqueeze(1)
                        .broadcast_to([P, OUT_F, IN_F])
                    )
                    nc.vector.tensor_mul(tmp[:], w1v, xv)
                    tmps.append(tmp)
                pending.append((po, tmps, 2 * pr))
                if len(pending) > 1:
                    emit_reduce(pending.pop(0))
            while pending:
                emit_reduce(pending.pop(0))
            nc.sync.dma_start(
                out=out_t[bass.ds(gb0, gsz), :].rearrange(
                    "(t p) o -> p t o", t=2 * grp
                ),
                in_=outs_g[:],
            )

        if loop and groups > 1:
            if reps == 1:
                with tc.For_i(0, groups * gsz, gsz) as iv:
                    emit_group(iv)
            else:
                with tc.For_i(0, reps, 1):
                    with tc.For_i(0, groups * gsz, gsz) as iv:
                        emit_group(iv)
        else:
            for it in range(groups * reps):
                emit_group((it % groups) * gsz)

    nc.compile()
    return nc


def _get_nc():
    global _cached_nc
    if _cached_nc is None:
        _cached_nc = _build_nc()
    return _cached_nc


def _make_in_maps(input, cond, cond_weight, cond_bias):
    ident = np.eye(P, dtype=np.float32)
    in_maps = []
    n_heads, b_total = input.shape[0], input.shape[1]
    for h in range(n_heads):
        cx = np.zeros((b_total, 64), np.float32)
        cx[:, :COND_IN] = cond[h]
        cx[:, COND_IN] = 1.0
        cx[:, C1 : C1 + IN_F] = input[h]
        cw3 = cond_weight[h].reshape(OUT_F, INP1, COND_IN)  # (o, i, c)
        cb2 = cond_bias[h].reshape(OUT_F, INP1)  # (o, i)
        cwk = np.zeros((P, KW), np.float32)
        cwk1 = cw3[:, :IN_F, :].transpose(2, 0, 1).reshape(COND_IN, KW)
        cwk[0:COND_IN] = cwk1
        cwk[COND_IN] = cb2[:, :IN_F].reshape(KW)
        cwk[64 : 64 + COND_IN] = cwk1
        cwk[64 + COND_IN] = cb2[:, :IN_F].reshape(KW)
        cwo = np.zeros((P, OUT_F), np.float32)
        cwo[0:COND_IN] = cw3[:, IN_F, :].T  # [c, o]
        cwo[COND_IN] = cb2[:, IN_F]
        cwo[64 : 64 + COND_IN] = cw3[:, IN_F, :].T
        cwo[64 + COND_IN] = cb2[:, IN_F]
        in_maps.append({"cx": cx, "cwk": cwk, "cwo": cwo, "ident": ident})
    return in_maps


def _run(in_maps, **kwargs):
    from concourse import bass_utils

    nc = _get_nc()
    return bass_utils.run_bass_kernel_spmd(
        nc, in_maps, core_ids=list(range(N_HEADS)), **kwargs
    )


def kernel(input, cond, cond_weight, cond_bias):
    input = np.asarray(input, np.float32)
    cond = np.asarray(cond, np.float32)
    cond_weight = np.asarray(cond_weight, np.float32)
    cond_bias = np.asarray(cond_bias, np.float32)
    in_maps = _make_in_maps(input, cond, cond_weight, cond_bias)
    res = _run(in_maps)
    return np.stack([r["out"] for r in res.results], axis=0)



# revision 10
# speedup vs baseline: 1.6703x; 1.6703x over previous
"""Trainium2 Bass kernel for nn_MetaMultiLinear.

Math (per head h, sample b):
    w[b, k]   = sum_c cond[b, c] * CW[k, c] + cb[k]        k = o*17 + i  (544)
    out[b, o] = sum_i x1[b, i] * w[b, o*17+i]              x1 = [input, 1] (17)

Sharding: head h -> NeuronCore h (8 heads, 8 cores), full B=32768 per core.

v2 design (vs v1): the host supplies cond1^T pre-transposed, so the
per-pair PE transpose + fence + PSUM->SBUF copy of v1 disappear.  The
i = 16 term (x1 = 1) and all bias terms flow through a small po matmul
(cond1 @ cwo) that opens each tile's PSUM accumulation group.

Per-core device algorithm, fully unrolled (256 tiles of 128 samples,
processed as 128 pairs; 8 groups of 16 pairs for DMA batching):
  1. Per group: two DMAs load cond1^T halves into partitions 0-32 and
     64-96 of one [128, 2048] tile (the split doubles DMA port
     coverage); one DMA loads x tiles sample-major.
  2. Per pair (PE, float32r): tile A uses the cond1^T slice at
     partitions 0-32 (tile_position (0,0)), tile B the one at 64-96
     ((64,0)).  For each: W-MM  w[b, o*16+i] = cond1 @ cwk^T (K=33,
     N=512, one PSUM bank) and po-MM po[b, o] = cond1 @ cwo (N=32,
     start=True, opens the accumulation group, carries all bias terms).
  3. Per pair (DVE, the floor): one tensor_mul over both PSUM banks:
     tmp[b, t, o, i] = w (*) broadcast(x), 1024 elem/partition.
  4. Per pair (PE, float32r): two reduce matmuls with identity
     stationary stream tmp i-outer/o-inner; the PSUM out AP is a
     broadcast view so 16 passes accumulate onto po via has_written.
     Reduces run one pair behind so the PE always has W work queued.
  5. ScalarE copies po -> SBUF (DMA cannot read PSUM); one output DMA
     per group; the host un-permutes tiles.
"""

import sys

import numpy as np

if "/opt/trn_rl_repo" not in sys.path:
    sys.path.insert(0, "/opt/trn_rl_repo")

N_HEADS, IN_F, COND_IN, OUT_F = 8, 16, 32, 32
B = 32768
INP1 = IN_F + 1  # 17
KW = OUT_F * IN_F  # 512 (i<16 part)
C1 = COND_IN + 1  # 33
P = 128
GROUPS = 8
PAIRS_PER_GROUP = B // (2 * P) // GROUPS  # 16
GCOLS = B // (2 * GROUPS)  # 2048 cond1T columns per group half

_cached_nc = None

USE_F32R = True
# "overlap": PE grouped reduce via overlapping PSUM out-AP (1 matmul/tile)
# "mm16":    PE grouped reduce via 16 accumulated strided matmuls (sim-safe)
REDUCE_MODE = "overlap"


def _build_nc():
    import concourse.mybir as mybir
    import concourse.tile as tile
    from concourse import bacc
    from contextlib import ExitStack

    f32 = mybir.dt.float32
    fr = mybir.dt.float32r if USE_F32R else f32
    nc = bacc.Bacc()

    # ct[r, g*2048+s]: r<33 -> cond1T[r, g*4096+s]; r>=33 -> cond1T[r-33, g*4096+2048+s]
    ct_t = nc.dram_tensor("ct", [2 * C1, GROUPS * GCOLS], fr, kind="ExternalInput")
    # x[p, ((g j) t) i] = input[g*4096 + t*2048 + j*128 + p, i]
    x_t = nc.dram_tensor("x", [P, B // P * IN_F], f32, kind="ExternalInput")
    # cwk[c, o*16+i] = CW[o*17+i, c] (i<16); row 32 = cond_bias slice; rows 64-96 repeat
    cwk_t = nc.dram_tensor("cwk", [P, KW], fr, kind="ExternalInput")
    # cwo[c, o] = CW[o*17+16, c]; row 32 = cond_bias[o*17+16]; rows 64-96 repeat
    cwo_t = nc.dram_tensor("cwo", [P, OUT_F], fr, kind="ExternalInput")
    ident_t = nc.dram_tensor("ident", [P, P], fr, kind="ExternalInput")
    # out[p, ((g j) t) o] = out[g*4096 + t*2048 + j*128 + p, o]
    out_t = nc.dram_tensor("out", [P, B // P * OUT_F], f32, kind="ExternalOutput")

    with tile.TileContext(nc) as tc, ExitStack() as ctx:
        consts = ctx.enter_context(tc.tile_pool(name="consts", bufs=1))
        pct = ctx.enter_context(tc.tile_pool(name="pct", bufs=2))
        px = ctx.enter_context(tc.tile_pool(name="px", bufs=2))
        pouts = ctx.enter_context(tc.tile_pool(name="pouts", bufs=2))
        ptmp = ctx.enter_context(tc.tile_pool(name="ptmp", bufs=3))
        ppw = ctx.enter_context(tc.tile_pool(name="ppw", bufs=2, space="PSUM"))
        ppo = ctx.enter_context(tc.tile_pool(name="ppo", bufs=2, space="PSUM"))

        cwk = consts.tile([P, KW], fr)
        nc.sync.dma_start(out=cwk, in_=cwk_t[:])
        cwo = consts.tile([P, OUT_F], fr)
        nc.sync.dma_start(out=cwo, in_=cwo_t[:])
        idn = consts.tile([P, P], fr)
        nc.sync.dma_start(out=idn, in_=ident_t[:])

        pending = []  # (po, tmp, outs_g, j)

        def emit_out_dma(outs_g, g):
            nc.sync.dma_start(
                out=out_t[
                    :,
                    g * PAIRS_PER_GROUP * 2 * OUT_F : (g + 1)
                    * PAIRS_PER_GROUP
                    * 2
                    * OUT_F,
                ].rearrange("p (j t o) -> p j t o", j=PAIRS_PER_GROUP, t=2),
                in_=outs_g[:],
            )

        def emit_reduce(item):
            po, tmp, outs_g, j, g = item
            for t in (0, 1):
                rhs = tmp[:, t].rearrange("p o i -> p i o")
                if REDUCE_MODE == "overlap":
                    # 16 streamed passes of 32 o-columns accumulate onto the
                    # same PSUM addresses via has_written. i-outer/o-inner
                    # keeps the dst innermost step-1/even/8B-aligned (fp32r
                    # paired PSUM write requirement).
                    ov = po[:, t, 0:OUT_F].unsqueeze(1).broadcast_to([P, IN_F, OUT_F])
                    nc.tensor.matmul(
                        ov, idn[:], rhs, start=False, stop=True, skip_group_check=True
                    )
                else:
                    for i in range(IN_F):
                        nc.tensor.matmul(
                            po[:, t, 0:OUT_F],
                            idn[:],
                            rhs[:, i, :],
                            start=False,
                            stop=(i == IN_F - 1),
                            skip_group_check=True,
                        )
            nc.scalar.copy(out=outs_g[:, j], in_=po[:, :, 0:OUT_F])
            if j == PAIRS_PER_GROUP - 1:
                emit_out_dma(outs_g, g)

        for g in range(GROUPS):
            ct_g = pct.tile([P, GCOLS], fr)
            nc.sync.dma_start(
                out=ct_g[0:C1, :], in_=ct_t[0:C1, g * GCOLS : (g + 1) * GCOLS]
            )
            nc.sync.dma_start(
                out=ct_g[64 : 64 + C1, :],
                in_=ct_t[C1 : 2 * C1, g * GCOLS : (g + 1) * GCOLS],
            )
            x_g = px.tile([P, PAIRS_PER_GROUP, 2, IN_F], f32)
            nc.sync.dma_start(
                out=x_g[:],
                in_=x_t[
                    :, g * PAIRS_PER_GROUP * 2 * IN_F : (g + 1) * PAIRS_PER_GROUP * 2 * IN_F
                ].rearrange("p (j t i) -> p j t i", j=PAIRS_PER_GROUP, t=2),
            )
            outs_g = pouts.tile([P, PAIRS_PER_GROUP, 2, OUT_F], f32)

            for j in range(PAIRS_PER_GROUP):
                wpair = ppw.tile([P, 2, KW], f32)
                po = ppo.tile([P, 2, 512], f32)
                for t, g0 in enumerate((0, 64)):
                    cts = ct_g[g0 : g0 + C1, j * P : (j + 1) * P]
                    nc.tensor.matmul(
                        wpair[:, t, :],
                        cts,
                        cwk[g0 : g0 + C1, :],
                        start=True,
                        stop=True,
                        tile_position=(g0, 0),
                    )
                    # opens tile t's accumulation group (own PSUM bank)
                    nc.tensor.matmul(
                        po[:, t, 0:OUT_F],
                        cts,
                        cwo[g0 : g0 + C1, :],
                        start=True,
                        stop=False,
                        skip_group_check=True,
                        tile_position=(g0, 0),
                    )
                wview = wpair[:].rearrange("p t (o i) -> p t o i", i=IN_F)
                xv = x_g[:, j].unsqueeze(2).broadcast_to([P, 2, OUT_F, IN_F])
                tmp = ptmp.tile([P, 2, OUT_F, IN_F], fr)
                nc.vector.tensor_mul(tmp[:], wview, xv)
                pending.append((po, tmp, outs_g, j, g))
                if len(pending) > 1:
                    emit_reduce(pending.pop(0))
        while pending:
            emit_reduce(pending.pop(0))

    nc.compile()
    return nc


def _get_nc():
    global _cached_nc
    if _cached_nc is None:
        _cached_nc = _build_nc()
    return _cached_nc


def _make_in_maps(input, cond, cond_weight, cond_bias):
    ident = np.eye(P, dtype=np.float32)
    in_maps = []
    n_heads, b_total = input.shape[0], input.shape[1]
    for h in range(n_heads):
        c1t = np.empty((C1, b_total), np.float32)
        c1t[:COND_IN] = cond[h].T
        c1t[COND_IN] = 1.0
        # [33, g, t, s] -> [t, 33, g, s] -> [66, g*s]
        ct = (
            c1t.reshape(C1, GROUPS, 2, GCOLS)
            .transpose(2, 0, 1, 3)
            .reshape(2 * C1, GROUPS * GCOLS)
        )
        ct = np.ascontiguousarray(ct)
        # x[p, (g j t i)] = input[g*4096 + t*2048 + j*128 + p, i]
        x = (
            input[h]
            .reshape(GROUPS, 2, PAIRS_PER_GROUP, P, IN_F)
            .transpose(3, 0, 2, 1, 4)
            .reshape(P, b_total // P * IN_F)
        )
        x = np.ascontiguousarray(x)
        cw3 = cond_weight[h].reshape(OUT_F, INP1, COND_IN)  # (o, i, c)
        cb2 = cond_bias[h].reshape(OUT_F, INP1)  # (o, i)
        cwk = np.zeros((P, KW), np.float32)
        cwk1 = cw3[:, :IN_F, :].transpose(2, 0, 1).reshape(COND_IN, KW)
        cwk[0:COND_IN] = cwk1
        cwk[COND_IN] = cb2[:, :IN_F].reshape(KW)
        cwk[64 : 64 + COND_IN] = cwk1
        cwk[64 + COND_IN] = cb2[:, :IN_F].reshape(KW)
        cwo = np.zeros((P, OUT_F), np.float32)
        cwo[0:COND_IN] = cw3[:, IN_F, :].T  # [c, o]
        cwo[COND_IN] = cb2[:, IN_F]
        cwo[64 : 64 + COND_IN] = cw3[:, IN_F, :].T
        cwo[64 + COND_IN] = cb2[:, IN_F]
        in_maps.append({"ct": ct, "x": x, "cwk": cwk, "cwo": cwo, "ident": ident})
    return in_maps


def _unpack_out(res):
    # out[p, (g j t o)] -> [g, t, j, p, o] -> [B, o]
    outs = []
    for r in res.results:
        o = (
            r["out"]
            .reshape(P, GROUPS, PAIRS_PER_GROUP, 2, OUT_F)
            .transpose(1, 3, 2, 0, 4)
            .reshape(B, OUT_F)
        )
        outs.append(o)
    return np.stack(outs, axis=0)


def _run(in_maps, **kwargs):
    from concourse import bass_utils

    nc = _get_nc()
    return bass_utils.run_bass_kernel_spmd(
        nc, in_maps, core_ids=list(range(N_HEADS)), **kwargs
    )


def kernel(input, cond, cond_weight, cond_bias):
    input = np.asarray(input, np.float32)
    cond = np.asarray(cond, np.float32)
    cond_weight = np.asarray(cond_weight, np.float32)
    cond_bias = np.asarray(cond_bias, np.float32)
    in_maps = _make_in_maps(input, cond, cond_weight, cond_bias)
    res = _run(in_maps)
    return _unpack_out(res)


# revision 19
# speedup vs baseline: 1.8296x; 1.0954x over previous
"""Trainium2 Bass kernel for nn_MetaMultiLinear.

Math (per head h, sample b):
    w[b, k]   = sum_c cond[b, c] * CW[k, c] + cb[k]        k = o*17 + i  (544)
    out[b, o] = sum_i x1[b, i] * w[b, o*17+i]              x1 = [input, 1] (17)

Sharding: head h -> NeuronCore h (8 heads, 8 cores), full B=32768 per core.

v3 design:
  - The host supplies cond1^T pre-transposed (two halves, at SBUF
    partitions 0-32 and 64-96), so no on-device transpose is needed.
  - Per tile of 128 samples (processed in pairs: tile A from partition
    half 0, tile B from half 64):  W-MM  w[b, o*16+i] = cond1 @ cwk^T
    (fp32r, K=33, N=512, one PSUM bank per tile) and po-MM
    po[b, o] = cond1 @ cwo (N=32, carries the bias and i=16 terms).
    Both po tiles share ONE PSUM bank: t=0 opens it (start=True,
    addresses 0:32), t=1 writes addresses 32:64 with start=False
    (per-element has_written: fresh addresses are overwritten, set
    addresses accumulate -- so no bank-wide clear wipes t=0).
  - DVE (the floor): one tensor_mul per pair over both W PSUM banks:
    tmp[b, t, o, i] = w (*) broadcast(x), 1024 elem/partition, output
    in bf16.
  - One reduce matmul per PAIR (bf16 moving operand allows N=1024):
    identity stationary streams tmp (t, i)-outer / o-inner; the PSUM
    out AP broadcasts over i so the 16 i-passes accumulate onto
    po[t*32+o] via has_written.  Reduces run one pair behind so the PE
    always has W work queued.
  - ScalarE copies po -> SBUF; one output DMA per group; the host
    un-permutes tiles.
"""

import sys

import numpy as np

if "/opt/trn_rl_repo" not in sys.path:
    sys.path.insert(0, "/opt/trn_rl_repo")

N_HEADS, IN_F, COND_IN, OUT_F = 8, 16, 32, 32
B = 32768
INP1 = IN_F + 1  # 17
KW = OUT_F * IN_F  # 512 (i<16 part)
C1 = COND_IN + 1  # 33
P = 128
GROUPS = 8
PAIRS_PER_GROUP = B // (2 * P) // GROUPS  # 16
GCOLS = B // (2 * GROUPS)  # 2048 cond1T columns per group half

_cached_nc = None

USE_F32R = True
# "overlap": PE grouped reduce via overlapping PSUM out-AP (1 matmul/pair)
# "mm16":    PE grouped reduce via 16 accumulated strided matmuls (sim-safe)
REDUCE_MODE = "overlap"


def _build_nc():
    import concourse.mybir as mybir
    import concourse.tile as tile
    from concourse import bacc
    from contextlib import ExitStack

    f32 = mybir.dt.float32
    bf16 = mybir.dt.bfloat16
    fr = mybir.dt.float32r if USE_F32R else f32
    nc = bacc.Bacc()

    # ct[r, g*2048+s]: r<33 -> cond1T[r, g*4096+s]; r>=33 -> cond1T[r-33, g*4096+2048+s]
    ct_t = nc.dram_tensor("ct", [2 * C1, GROUPS * GCOLS], fr, kind="ExternalInput")
    # x[p, ((g j) t) i] = input[g*4096 + t*2048 + j*128 + p, i]
    x_t = nc.dram_tensor("x", [P, B // P * IN_F], f32, kind="ExternalInput")
    # cwk[c, o*16+i] = CW[o*17+i, c] (i<16); row 32 = cond_bias slice; rows 64-96 repeat
    cwk_t = nc.dram_tensor("cwk", [P, KW], fr, kind="ExternalInput")
    # cwo[c, o] = CW[o*17+16, c]; row 32 = cond_bias[o*17+16]; rows 64-96 repeat
    cwo_t = nc.dram_tensor("cwo", [P, OUT_F], fr, kind="ExternalInput")
    ident_t = nc.dram_tensor("ident", [P, P], bf16, kind="ExternalInput")
    # out[p, ((g j) t) o] = out[g*4096 + t*2048 + j*128 + p, o]
    out_t = nc.dram_tensor("out", [P, B // P * OUT_F], f32, kind="ExternalOutput")

    with tile.TileContext(nc) as tc, ExitStack() as ctx:
        consts = ctx.enter_context(tc.tile_pool(name="consts", bufs=1))
        pct = ctx.enter_context(tc.tile_pool(name="pct", bufs=2))
        px = ctx.enter_context(tc.tile_pool(name="px", bufs=2))
        pouts = ctx.enter_context(tc.tile_pool(name="pouts", bufs=2))
        ptmp = ctx.enter_context(tc.tile_pool(name="ptmp", bufs=3))
        ppw = ctx.enter_context(tc.tile_pool(name="ppw", bufs=2, space="PSUM"))
        ppo = ctx.enter_context(tc.tile_pool(name="ppo", bufs=2, space="PSUM"))

        cwk = consts.tile([P, KW], fr)
        nc.sync.dma_start(out=cwk, in_=cwk_t[:])
        cwo = consts.tile([P, OUT_F], fr)
        nc.sync.dma_start(out=cwo, in_=cwo_t[:])
        idn = consts.tile([P, P], bf16)
        nc.sync.dma_start(out=idn, in_=ident_t[:])

        pending = []  # (po, tmp, outs_g, j, g)

        def emit_out_dma(outs_g, g):
            nc.sync.dma_start(
                out=out_t[
                    :,
                    g * PAIRS_PER_GROUP * 2 * OUT_F : (g + 1)
                    * PAIRS_PER_GROUP
                    * 2
                    * OUT_F,
                ].rearrange("p (j t o) -> p j t o", j=PAIRS_PER_GROUP, t=2),
                in_=outs_g[:],
            )

        def emit_reduce(item):
            po, tmp, outs_g, j, g = item
            for t in (0, 1):
                # i-outer / o-inner, fully contiguous rhs; 16 passes of 32
                # o-columns accumulate onto po[t*32+o] via has_written.
                rhs = tmp[:, t]
                if REDUCE_MODE == "overlap":
                    ov = (
                        po[:, t, 0:OUT_F]
                        .unsqueeze(1)
                        .broadcast_to([P, IN_F, OUT_F])
                    )
                    nc.tensor.matmul(
                        ov,
                        idn[:],
                        rhs,
                        start=False,
                        stop=True,
                        skip_group_check=True,
                    )
                else:
                    for i in range(IN_F):
                        nc.tensor.matmul(
                            po[:, t, 0:OUT_F],
                            idn[:],
                            rhs[:, i, :],
                            start=False,
                            stop=(i == IN_F - 1),
                            skip_group_check=True,
                        )
            nc.scalar.copy(out=outs_g[:, j], in_=po[:, :, 0:OUT_F])
            if j == PAIRS_PER_GROUP - 1:
                emit_out_dma(outs_g, g)

        for g in range(GROUPS):
            ct_g = pct.tile([P, GCOLS], fr)
            nc.sync.dma_start(
                out=ct_g[0:C1, :], in_=ct_t[0:C1, g * GCOLS : (g + 1) * GCOLS]
            )
            nc.sync.dma_start(
                out=ct_g[64 : 64 + C1, :],
                in_=ct_t[C1 : 2 * C1, g * GCOLS : (g + 1) * GCOLS],
            )
            x_g = px.tile([P, PAIRS_PER_GROUP, 2, IN_F], f32)
            nc.sync.dma_start(
                out=x_g[:],
                in_=x_t[
                    :,
                    g * PAIRS_PER_GROUP * 2 * IN_F : (g + 1)
                    * PAIRS_PER_GROUP
                    * 2
                    * IN_F,
                ].rearrange("p (j t i) -> p j t i", j=PAIRS_PER_GROUP, t=2),
            )
            outs_g = pouts.tile([P, PAIRS_PER_GROUP, 2, OUT_F], f32)

            for j in range(PAIRS_PER_GROUP):
                wpair = ppw.tile([P, 2, KW], f32)
                po = ppo.tile([P, 2, 512], f32)
                for t, g0 in enumerate((0, 64)):
                    cts = ct_g[g0 : g0 + C1, j * P : (j + 1) * P]
                    nc.tensor.matmul(
                        wpair[:, t, :],
                        cts,
                        cwk[g0 : g0 + C1, :],
                        start=True,
                        stop=True,
                        tile_position=(g0, 0),
                    )
                    # Both po tiles in ONE bank: t=0 start=True clears the
                    # bank and sets has_written for 0:32; t=1 start=False
                    # overwrites the still-clear 32:64 without clearing.
                    nc.tensor.matmul(
                        po[:, t, 0:OUT_F],
                        cts,
                        cwo[g0 : g0 + C1, :],
                        start=True,
                        stop=False,
                        skip_group_check=True,
                        tile_position=(g0, 0),
                    )
                tmp = ptmp.tile([P, 2, IN_F, OUT_F], bf16)
                wview = wpair[:].rearrange("p t (o i) -> p t i o", i=IN_F)
                xv = x_g[:, j].unsqueeze(3).broadcast_to([P, 2, IN_F, OUT_F])
                nc.vector.tensor_mul(tmp[:], wview, xv)
                pending.append((po, tmp, outs_g, j, g))
                if len(pending) > 1:
                    emit_reduce(pending.pop(0))
        while pending:
            emit_reduce(pending.pop(0))

    nc.compile()
    return nc


def _get_nc():
    global _cached_nc
    if _cached_nc is None:
        _cached_nc = _build_nc()
    return _cached_nc


def _make_in_maps(input, cond, cond_weight, cond_bias):
    import ml_dtypes

    ident = np.eye(P, dtype=ml_dtypes.bfloat16)
    in_maps = []
    n_heads, b_total = input.shape[0], input.shape[1]
    for h in range(n_heads):
        c1t = np.empty((C1, b_total), np.float32)
        c1t[:COND_IN] = cond[h].T
        c1t[COND_IN] = 1.0
        # [33, g, t, s] -> [t, 33, g, s] -> [66, g*s]
        ct = (
            c1t.reshape(C1, GROUPS, 2, GCOLS)
            .transpose(2, 0, 1, 3)
            .reshape(2 * C1, GROUPS * GCOLS)
        )
        ct = np.ascontiguousarray(ct)
        # x[p, (g j t i)] = input[g*4096 + t*2048 + j*128 + p, i]
        x = (
            input[h]
            .reshape(GROUPS, 2, PAIRS_PER_GROUP, P, IN_F)
            .transpose(3, 0, 2, 1, 4)
            .reshape(P, b_total // P * IN_F)
        )
        x = np.ascontiguousarray(x)
        cw3 = cond_weight[h].reshape(OUT_F, INP1, COND_IN)  # (o, i, c)
        cb2 = cond_bias[h].reshape(OUT_F, INP1)  # (o, i)
        cwk = np.zeros((P, KW), np.float32)
        cwk1 = cw3[:, :IN_F, :].transpose(2, 0, 1).reshape(COND_IN, KW)
        cwk[0:COND_IN] = cwk1
        cwk[COND_IN] = cb2[:, :IN_F].reshape(KW)
        cwk[64 : 64 + COND_IN] = cwk1
        cwk[64 + COND_IN] = cb2[:, :IN_F].reshape(KW)
        cwo = np.zeros((P, OUT_F), np.float32)
        cwo[0:COND_IN] = cw3[:, IN_F, :].T  # [c, o]
        cwo[COND_IN] = cb2[:, IN_F]
        cwo[64 : 64 + COND_IN] = cw3[:, IN_F, :].T
        cwo[64 + COND_IN] = cb2[:, IN_F]
        in_maps.append({"ct": ct, "x": x, "cwk": cwk, "cwo": cwo, "ident": ident})
    return in_maps


def _unpack_out(res):
    # out[p, (g j t o)] -> [g, t, j, p, o] -> [B, o]
    outs = []
    for r in res.results:
        o = (
            r["out"]
            .reshape(P, GROUPS, PAIRS_PER_GROUP, 2, OUT_F)
            .transpose(1, 3, 2, 0, 4)
            .reshape(B, OUT_F)
        )
        outs.append(o)
    return np.stack(outs, axis=0)


def _run(in_maps, **kwargs):
    from concourse import bass_utils

    nc = _get_nc()
    return bass_utils.run_bass_kernel_spmd(
        nc, in_maps, core_ids=list(range(N_HEADS)), **kwargs
    )


def kernel(input, cond, cond_weight, cond_bias):
    input = np.asarray(input, np.float32)
    cond = np.asarray(cond, np.float32)
    cond_weight = np.asarray(cond_weight, np.float32)
    cond_bias = np.asarray(cond_bias, np.float32)
    in_maps = _make_in_maps(input, cond, cond_weight, cond_bias)
    res = _run(in_maps)
    return _unpack_out(res)


# revision 20
# speedup vs baseline: 2.0359x; 1.1127x over previous
"""Trainium2 Bass kernel for nn_MetaMultiLinear.

Math (per head h, sample b):
    w[b, k]   = sum_c cond[b, c] * CW[k, c] + cb[k]        k = o*17 + i  (544)
    out[b, o] = sum_i x1[b, i] * w[b, o*17+i]              x1 = [input, 1] (17)

Sharding: head h -> NeuronCore h (8 heads, 8 cores), full B=32768 per core.

v3 design:
  - The host supplies cond1^T pre-transposed (two halves, at SBUF
    partitions 0-32 and 64-96), so no on-device transpose is needed.
  - Per tile of 128 samples (processed in pairs: tile A from partition
    half 0, tile B from half 64):  W-MM  w[b, o*16+i] = cond1 @ cwk^T
    (fp32r, K=33, N=512, one PSUM bank per tile) and po-MM
    po[b, o] = cond1 @ cwo (N=32, carries the bias and i=16 terms).
    Both po tiles share ONE PSUM bank: t=0 opens it (start=True,
    addresses 0:32), t=1 writes addresses 32:64 with start=False
    (per-element has_written: fresh addresses are overwritten, set
    addresses accumulate -- so no bank-wide clear wipes t=0).
  - DVE (the floor): one tensor_mul per pair over both W PSUM banks:
    tmp[b, t, o, i] = w (*) broadcast(x), 1024 elem/partition, output
    in bf16.
  - One reduce matmul per PAIR (bf16 moving operand allows N=1024):
    identity stationary streams tmp (t, i)-outer / o-inner; the PSUM
    out AP broadcasts over i so the 16 i-passes accumulate onto
    po[t*32+o] via has_written.  Reduces run one pair behind so the PE
    always has W work queued.
  - ScalarE copies po -> SBUF; one output DMA per group; the host
    un-permutes tiles.
"""

import sys

import numpy as np

if "/opt/trn_rl_repo" not in sys.path:
    sys.path.insert(0, "/opt/trn_rl_repo")

N_HEADS, IN_F, COND_IN, OUT_F = 8, 16, 32, 32
B = 32768
INP1 = IN_F + 1  # 17
KW = OUT_F * IN_F  # 512 (i<16 part)
C1 = COND_IN + 1  # 33
P = 128
GROUPS = 8
PAIRS_PER_GROUP = B // (2 * P) // GROUPS  # 16
GCOLS = B // (2 * GROUPS)  # 2048 cond1T columns per group half

_cached_nc = None

USE_F32R = True
# "overlap": PE grouped reduce via overlapping PSUM out-AP (1 matmul/pair)
# "mm16":    PE grouped reduce via 16 accumulated strided matmuls (sim-safe)
REDUCE_MODE = "overlap"


def _build_nc():
    import concourse.mybir as mybir
    import concourse.tile as tile
    from concourse import bacc
    from contextlib import ExitStack

    f32 = mybir.dt.float32
    bf16 = mybir.dt.bfloat16
    fr = mybir.dt.float32r if USE_F32R else f32
    nc = bacc.Bacc()

    # ct[r, g*2048+s]: r<33 -> cond1T[r, g*4096+s]; r>=33 -> cond1T[r-33, g*4096+2048+s]
    ct_t = nc.dram_tensor("ct", [2 * C1, GROUPS * GCOLS], bf16, kind="ExternalInput")
    # x[p, ((g j) t) i] = input[g*4096 + t*2048 + j*128 + p, i]
    x_t = nc.dram_tensor("x", [P, B // P * IN_F], f32, kind="ExternalInput")
    # cwk[c, o*16+i] = CW[o*17+i, c] (i<16); row 32 = cond_bias slice; rows 64-96 repeat
    cwk_t = nc.dram_tensor("cwk", [P, KW], bf16, kind="ExternalInput")
    # cwo[c, o] = CW[o*17+16, c]; row 32 = cond_bias[o*17+16]; rows 64-96 repeat
    cwo_t = nc.dram_tensor("cwo", [P, OUT_F], bf16, kind="ExternalInput")
    ident_t = nc.dram_tensor("ident", [P, P], bf16, kind="ExternalInput")
    # out[p, ((g j) t) o] = out[g*4096 + t*2048 + j*128 + p, o]
    out_t = nc.dram_tensor("out", [P, B // P * OUT_F], f32, kind="ExternalOutput")

    with tile.TileContext(nc) as tc, ExitStack() as ctx:
        consts = ctx.enter_context(tc.tile_pool(name="consts", bufs=1))
        pct = ctx.enter_context(tc.tile_pool(name="pct", bufs=2))
        px = ctx.enter_context(tc.tile_pool(name="px", bufs=2))
        pouts = ctx.enter_context(tc.tile_pool(name="pouts", bufs=2))
        ptmp = ctx.enter_context(tc.tile_pool(name="ptmp", bufs=3))
        ppw = ctx.enter_context(tc.tile_pool(name="ppw", bufs=2, space="PSUM"))
        ppo = ctx.enter_context(tc.tile_pool(name="ppo", bufs=2, space="PSUM"))

        cwk = consts.tile([P, KW], bf16)
        nc.sync.dma_start(out=cwk, in_=cwk_t[:])
        cwo = consts.tile([P, OUT_F], bf16)
        nc.sync.dma_start(out=cwo, in_=cwo_t[:])
        idn = consts.tile([P, P], bf16)
        nc.sync.dma_start(out=idn, in_=ident_t[:])

        pending = []  # (po, tmp, outs_g, j, g)

        def emit_out_dma(outs_g, g):
            nc.sync.dma_start(
                out=out_t[
                    :,
                    g * PAIRS_PER_GROUP * 2 * OUT_F : (g + 1)
                    * PAIRS_PER_GROUP
                    * 2
                    * OUT_F,
                ].rearrange("p (j t o) -> p j t o", j=PAIRS_PER_GROUP, t=2),
                in_=outs_g[:],
            )

        def emit_reduce(item):
            po, tmp, outs_g, j, g = item
            for t in (0, 1):
                # i-outer / o-inner, fully contiguous rhs; 16 passes of 32
                # o-columns accumulate onto po[t*32+o] via has_written.
                rhs = tmp[:, t]
                if REDUCE_MODE == "overlap":
                    ov = (
                        po[:, t, 0:OUT_F]
                        .unsqueeze(1)
                        .broadcast_to([P, IN_F, OUT_F])
                    )
                    nc.tensor.matmul(
                        ov,
                        idn[:],
                        rhs,
                        start=False,
                        stop=True,
                        skip_group_check=True,
                    )
                else:
                    for i in range(IN_F):
                        nc.tensor.matmul(
                            po[:, t, 0:OUT_F],
                            idn[:],
                            rhs[:, i, :],
                            start=False,
                            stop=(i == IN_F - 1),
                            skip_group_check=True,
                        )
            nc.scalar.copy(out=outs_g[:, j], in_=po[:, :, 0:OUT_F])
            if j == PAIRS_PER_GROUP - 1:
                emit_out_dma(outs_g, g)

        for g in range(GROUPS):
            ct_g = pct.tile([P, GCOLS], bf16)
            nc.sync.dma_start(
                out=ct_g[0:C1, :], in_=ct_t[0:C1, g * GCOLS : (g + 1) * GCOLS]
            )
            nc.sync.dma_start(
                out=ct_g[64 : 64 + C1, :],
                in_=ct_t[C1 : 2 * C1, g * GCOLS : (g + 1) * GCOLS],
            )
            x_g = px.tile([P, PAIRS_PER_GROUP, 2, IN_F], f32)
            nc.sync.dma_start(
                out=x_g[:],
                in_=x_t[
                    :,
                    g * PAIRS_PER_GROUP * 2 * IN_F : (g + 1)
                    * PAIRS_PER_GROUP
                    * 2
                    * IN_F,
                ].rearrange("p (j t i) -> p j t i", j=PAIRS_PER_GROUP, t=2),
            )
            outs_g = pouts.tile([P, PAIRS_PER_GROUP, 2, OUT_F], f32)

            for j in range(PAIRS_PER_GROUP):
                wpair = ppw.tile([P, 2, KW], f32)
                po = ppo.tile([P, 2, 512], f32)
                for t, g0 in enumerate((0, 64)):
                    cts = ct_g[g0 : g0 + C1, j * P : (j + 1) * P]
                    nc.tensor.matmul(
                        wpair[:, t, :],
                        cts,
                        cwk[g0 : g0 + C1, :],
                        start=True,
                        stop=True,
                        tile_position=(g0, 0),
                    )
                    # Both po tiles in ONE bank: t=0 start=True clears the
                    # bank and sets has_written for 0:32; t=1 start=False
                    # overwrites the still-clear 32:64 without clearing.
                    nc.tensor.matmul(
                        po[:, t, 0:OUT_F],
                        cts,
                        cwo[g0 : g0 + C1, :],
                        start=True,
                        stop=False,
                        skip_group_check=True,
                        tile_position=(g0, 0),
                    )
                tmp = ptmp.tile([P, 2, IN_F, OUT_F], bf16)
                wview = wpair[:].rearrange("p t (o i) -> p t i o", i=IN_F)
                xv = x_g[:, j].unsqueeze(3).broadcast_to([P, 2, IN_F, OUT_F])
                nc.vector.tensor_mul(tmp[:], wview, xv)
                pending.append((po, tmp, outs_g, j, g))
                if len(pending) > 1:
                    emit_reduce(pending.pop(0))
        while pending:
            emit_reduce(pending.pop(0))

    nc.compile()
    return nc


def _get_nc():
    global _cached_nc
    if _cached_nc is None:
        _cached_nc = _build_nc()
    return _cached_nc


def _make_in_maps(input, cond, cond_weight, cond_bias):
    import ml_dtypes

    bf = ml_dtypes.bfloat16
    ident = np.eye(P, dtype=bf)
    in_maps = []
    n_heads, b_total = input.shape[0], input.shape[1]
    for h in range(n_heads):
        c1t = np.empty((C1, b_total), np.float32)
        c1t[:COND_IN] = cond[h].T
        c1t[COND_IN] = 1.0
        # [33, g, t, s] -> [t, 33, g, s] -> [66, g*s]
        ct = (
            c1t.reshape(C1, GROUPS, 2, GCOLS)
            .transpose(2, 0, 1, 3)
            .reshape(2 * C1, GROUPS * GCOLS)
        )
        ct = np.ascontiguousarray(ct)
        # x[p, (g j t i)] = input[g*4096 + t*2048 + j*128 + p, i]
        x = (
            input[h]
            .reshape(GROUPS, 2, PAIRS_PER_GROUP, P, IN_F)
            .transpose(3, 0, 2, 1, 4)
            .reshape(P, b_total // P * IN_F)
        )
        x = np.ascontiguousarray(x)
        cw3 = cond_weight[h].reshape(OUT_F, INP1, COND_IN)  # (o, i, c)
        cb2 = cond_bias[h].reshape(OUT_F, INP1)  # (o, i)
        cwk = np.zeros((P, KW), np.float32)
        cwk1 = cw3[:, :IN_F, :].transpose(2, 0, 1).reshape(COND_IN, KW)
        cwk[0:COND_IN] = cwk1
        cwk[COND_IN] = cb2[:, :IN_F].reshape(KW)
        cwk[64 : 64 + COND_IN] = cwk1
        cwk[64 + COND_IN] = cb2[:, :IN_F].reshape(KW)
        cwo = np.zeros((P, OUT_F), np.float32)
        cwo[0:COND_IN] = cw3[:, IN_F, :].T  # [c, o]
        cwo[COND_IN] = cb2[:, IN_F]
        cwo[64 : 64 + COND_IN] = cw3[:, IN_F, :].T
        cwo[64 + COND_IN] = cb2[:, IN_F]
        in_maps.append(
            {
                "ct": ct.astype(bf),
                "x": x,
                "cwk": cwk.astype(bf),
                "cwo": cwo.astype(bf),
                "ident": ident,
            }
        )
    return in_maps


def _unpack_out(res):
    # out[p, (g j t o)] -> [g, t, j, p, o] -> [B, o]
    outs = []
    for r in res.results:
        o = (
            r["out"]
            .reshape(P, GROUPS, PAIRS_PER_GROUP, 2, OUT_F)
            .transpose(1, 3, 2, 0, 4)
            .reshape(B, OUT_F)
        )
        outs.append(o)
    return np.stack(outs, axis=0)


def _run(in_maps, **kwargs):
    from concourse import bass_utils

    nc = _get_nc()
    return bass_utils.run_bass_kernel_spmd(
        nc, in_maps, core_ids=list(range(N_HEADS)), **kwargs
    )


def kernel(input, cond, cond_weight, cond_bias):
    input = np.asarray(input, np.float32)
    cond = np.asarray(cond, np.float32)
    cond_weight = np.asarray(cond_weight, np.float32)
    cond_bias = np.asarray(cond_bias, np.float32)
    in_maps = _make_in_maps(input, cond, cond_weight, cond_bias)
    res = _run(in_maps)
    return _unpack_out(res)
